# revision 1
# baseline (speedup 1.0000x reference)
"""GAT (3-layer, DGL-style) forward pass on 8 Trainium2 NeuronCores.

Strategy (dst-node sharded, graph-parallel):
  - Nodes are partitioned into 8 contiguous shards (dst ownership); edges are
    grouped by owner(dst), then by 128-node dst tile, then by table quarter
    (int16 gather-index range).
  - Per layer, each core computes feat_aug = h_shard @ [W | W@Ar] for its own
    shard; feat goes out as a bf16 row table via AllGather, the er projection
    stays local (only own-dst er is ever needed).
  - Edge phase per dst tile: src feature rows are fetched with dma_gather
    (bf16 256B rows, 4 SWDGE queues in parallel), el is recomputed on the fly
    from the gathered features, er is expanded edge-wise with a
    host-precomputed transposed one-hot matmul, and
    ex = exp(leaky_relu(el+er)) weights the features. One-hot matmuls
    accumulate sum(ex*feat) and sum(ex) per dst node in PSUM (the softmax
    max-subtraction cancels and is skipped; logits are O(1)).
  - Epilogue: normalize, residual, bias, ELU, plus the next layer's dense
    matmul fused in via an on-chip transpose (layer 3: projected residual and
    the final classifier are fused the same way).

All core-dependent information lives in per-core input tensors, so every core
runs an identical SPMD program.
"""

import math
import sys

import numpy as np

for _p in ("/opt/trn_rl_repo",):
    if _p not in sys.path:
        sys.path.insert(0, _p)

import ml_dtypes

BF16 = ml_dtypes.bfloat16

P = 128
NEG_SLOPE = 0.2
HID = 32
N_CLS = 40
N_CORES = 8
NQUARTER = 4
NI_HW_MAX = 1024  # dma_gather num_idxs beyond this crashes (HW-probed)

_PROGRAM_CACHE = {}


# ----------------------------------------------------------------------------
# Host-side preparation (index manipulation / sharding only)
# ----------------------------------------------------------------------------

def _make_waug(W, al, ar, with_al):
    """[in, H*D] -> [in, H*D (+H if with_al) +H] f32: [W | (W@Al) | W@Ar]."""
    H, D = al.shape
    W64 = W.astype(np.float64)

    def proj(a):
        A = np.zeros((H * D, H), np.float64)
        A[np.arange(H * D), np.arange(H * D) // D] = a.reshape(-1).astype(np.float64)
        return W64 @ A

    parts = [W64]
    if with_al:
        parts.append(proj(al))
    parts.append(proj(ar))
    return np.concatenate(parts, axis=1).astype(np.float32)


def _prepare(x, src, dst, n_cores=N_CORES):
    n_nodes = x.shape[0]
    assert n_nodes % n_cores == 0
    shard = n_nodes // n_cores
    shard_pad = ((shard + P - 1) // P) * P
    T = shard_pad // P
    qrows = (n_cores * shard_pad) // NQUARTER
    assert qrows <= 32767, "quarter must fit int16 index range"

    owner = dst // shard
    local = dst - owner * shard
    tloc = local // P
    doff = (local % P).astype(np.int16)
    srcrow = ((src // shard) * shard_pad + (src % shard)).astype(np.int32)
    quarter = srcrow // qrows
    qidx = (srcrow % qrows).astype(np.int16)

    group = (owner * T + tloc) * NQUARTER + quarter  # [E]
    n_groups = n_cores * T * NQUARTER
    counts = np.bincount(group, minlength=n_groups).reshape(n_cores, T, NQUARTER)
    ni_tq = counts.max(axis=0)  # [T, NQ] shared across cores (SPMD)
    assert ni_tq.max() <= NI_HW_MAX, ni_tq.max()
    ch_tq = (ni_tq + P - 1) // P  # chunks per (tile, quarter)
    nch_t = ch_tq.sum(axis=1)  # [T]
    NCHMAX = int(nch_t.max())
    b0_tq = np.concatenate(
        [np.zeros((T, 1), np.int64), np.cumsum(ch_tq, axis=1)[:, :3]], axis=1
    )
    ni16_tq = ((ni_tq + 15) // 16) * 16
    ic_tq = ni16_tq // 16  # idx col counts
    icol0_tq = np.concatenate(
        [np.zeros((T, 1), np.int64), np.cumsum(ic_tq, axis=1)[:, :3]], axis=1
    )
    ICW = int((ic_tq.sum(axis=1)).max())

    # order edges by (core, tile, quarter); position within group
    order = np.argsort(group, kind="stable")
    g_sorted = group[order]
    starts = np.zeros(n_groups + 1, np.int64)
    np.cumsum(np.bincount(group, minlength=n_groups), out=starts[1:])
    pos = np.arange(len(order)) - starts[g_sorted]

    gidx_all = np.zeros((n_cores, T, ICW * 16), np.int16)
    dst_pb = np.full((n_cores, T, P, NCHMAX), -1, np.int16)

    oc = g_sorted // (T * NQUARTER)
    tc = (g_sorted // NQUARTER) % T
    qc = g_sorted % NQUARTER
    # gather-index stream position: per (t,q) block of 16-padded length
    gpos = icol0_tq[tc, qc] * 16 + pos
    gidx_all[oc, tc, gpos] = qidx[order]
    # slot (within tile): quarter base chunk * 128 + pos
    slot = b0_tq[tc, qc] * P + pos
    dst_pb[oc, tc, slot % P, slot // P] = doff[order]

    # wrap gather indices: index i of a block at [i%16, i//16], replicated x8
    gidx_w = gidx_all.reshape(n_cores, T, ICW, 16).transpose(0, 1, 3, 2)  # [.,16,ICW]
    gidx_w = np.broadcast_to(gidx_w[:, :, None, :, :],
                             (n_cores, T, 8, 16, ICW))
    gidx_w = np.ascontiguousarray(gidx_w).reshape(n_cores, T * P, ICW)

    # transposed one-hot for er expansion: ohT[t, i, b*128+p] = (dst_pb==i)
    i_ar = np.arange(P, dtype=np.int16)
    oht = (dst_pb[:, :, None, :, :] == i_ar[None, None, :, None, None])
    # [C, T, i, p, b] -> [C, T, i, b, p]
    oht = oht.transpose(0, 1, 2, 4, 3).astype(BF16)
    oht = np.ascontiguousarray(oht.reshape(n_cores, T * P, NCHMAX * P))

    dstoff_bf = dst_pb.astype(BF16)  # [-1 or 0..127] exact in bf16
    dstoff_i16 = dstoff_bf.view(np.int16).reshape(n_cores, T * P, NCHMAX)

    xT_per_core = []
    for c in range(n_cores):
        xs = x[c * shard:(c + 1) * shard].astype(np.float32)
        if shard_pad != shard:
            xs = np.concatenate(
                [xs, np.zeros((shard_pad - shard, xs.shape[1]), np.float32)], 0)
        xT_per_core.append(np.ascontiguousarray(xs.T))

    return dict(
        shard=shard, shard_pad=shard_pad, T=T, qrows=qrows,
        NCHMAX=NCHMAX, ICW=ICW,
        ni_tq=ni_tq.tolist(), ch_tq=ch_tq.tolist(), nch_t=nch_t.tolist(),
        b0_tq=b0_tq.tolist(), icol0_tq=icol0_tq.tolist(),
        ic_tq=ic_tq.tolist(),
        gidx_per_core=[np.ascontiguousarray(gidx_w[c]) for c in range(n_cores)],
        dstoff_per_core=[np.ascontiguousarray(dstoff_i16[c]) for c in range(n_cores)],
        oht_per_core=[np.ascontiguousarray(oht[c]) for c in range(n_cores)],
        xT_per_core=xT_per_core,
    )


# ----------------------------------------------------------------------------
# Device program
# ----------------------------------------------------------------------------

def _build_program(n_cores, plan, has_bias):
    from concourse import bacc, bass, tile
    import concourse.mybir as mybir
    from concourse.masks import make_identity

    dt = mybir.dt
    f32, bf16, i16, i32 = dt.float32, dt.bfloat16, dt.int16, dt.int32
    Alu = mybir.AluOpType
    Act = mybir.ActivationFunctionType

    shard, SP, T = plan["shard"], plan["shard_pad"], plan["T"]
    QROWS = plan["qrows"]
    NCHMAX, ICW = plan["NCHMAX"], plan["ICW"]
    ni_tq, ch_tq = plan["ni_tq"], plan["ch_tq"]
    nch_t, b0_tq = plan["nch_t"], plan["b0_tq"]
    icol0_tq, ic_tq = plan["icol0_tq"], plan["ic_tq"]
    rg = [list(range(n_cores))]

    nc = bacc.Bacc("TRN2", target_bir_lowering=False, debug=False,
                   num_devices=n_cores, num_swdge_queues=4)

    xT = nc.dram_tensor("xT", [P, SP], f32, kind="ExternalInput")
    gidx = nc.dram_tensor("gidx", [T * P, ICW], i16, kind="ExternalInput")
    dstoff = nc.dram_tensor("dstoff", [T * P, NCHMAX], i16, kind="ExternalInput")
    ohtd = nc.dram_tensor("ohtd", [T * P, NCHMAX * P], bf16, kind="ExternalInput")
    waug1 = nc.dram_tensor("waug1", [P, 132], f32, kind="ExternalInput")
    waug2 = nc.dram_tensor("waug2", [P, 132], f32, kind="ExternalInput")
    waug3 = nc.dram_tensor("waug3", [P, P], f32, kind="ExternalInput")
    al1r = nc.dram_tensor("al1r", [P, P], bf16, kind="ExternalInput")
    al2r = nc.dram_tensor("al2r", [P, P], bf16, kind="ExternalInput")
    res3w = nc.dram_tensor("res3w", [P, HID], f32, kind="ExternalInput")
    wfc = nc.dram_tensor("wfc", [HID, N_CLS], f32, kind="ExternalInput")
    bias_d = [None] * 4
    bias_shapes = [(P, P), (P, P), (P, HID), (P, N_CLS)]
    for i, hb in enumerate(has_bias):
        if hb:
            bias_d[i] = nc.dram_tensor(f"bias{i}", list(bias_shapes[i]), f32,
                                       kind="ExternalInput")
    out_e = nc.dram_tensor("out", [shard, N_CLS], f32, kind="ExternalOutput")

    agin = [nc.dram_tensor(f"agin{l}", [SP, P], bf16, kind="Internal")
            for l in range(3)]
    tables = [nc.dram_tensor(f"table{l}", [n_cores * SP, P], bf16,
                             kind="Internal", addr_space="Shared")
              for l in range(3)]
    h1d = nc.dram_tensor("h1d", [SP, P], f32, kind="Internal")

    with tile.TileContext(nc) as tc:
        with (
            tc.tile_pool(name="const", bufs=1) as cpool,
            tc.tile_pool(name="big", bufs=1) as bigpool,
            tc.tile_pool(name="gth", bufs=3) as gpool,
            tc.tile_pool(name="oht", bufs=3) as opool,
            tc.tile_pool(name="work", bufs=3) as wpool,
            tc.tile_pool(name="wsm", bufs=3) as spool,
            tc.tile_pool(name="pagg", bufs=2, space="PSUM") as p_agg,
            tc.tile_pool(name="ptr", bufs=2, space="PSUM") as p_tr,
            tc.tile_pool(name="pdn", bufs=2, space="PSUM") as p_dn,
            tc.tile_pool(name="per", bufs=1, space="PSUM") as p_er,
            tc.tile_pool(name="prs", bufs=1, space="PSUM") as p_rs,
        ):
            ident = cpool.tile([P, P], f32)
            make_identity(nc, ident[:])
            iota_i = cpool.tile([P, P], i32)
            nc.gpsimd.iota(iota_i[:], pattern=[[1, P]], base=0, channel_multiplier=0)
            iota_bf = cpool.tile([P, P], bf16)
            nc.vector.tensor_copy(iota_bf[:], iota_i[:])

            w1_sb = cpool.tile([P, 132], f32)
            nc.sync.dma_start(w1_sb[:], waug1[:, :])
            w2_sb = cpool.tile([P, 132], f32)
            nc.sync.dma_start(w2_sb[:], waug2[:, :])
            w3_sb = cpool.tile([P, P], f32)
            nc.sync.dma_start(w3_sb[:], waug3[:, :])
            al_sb = [cpool.tile([P, P], bf16, name=f"al{i}_sb") for i in range(2)]
            nc.sync.dma_start(al_sb[0][:], al1r[:, :])
            nc.sync.dma_start(al_sb[1][:], al2r[:, :])
            res3_sb = cpool.tile([P, HID], f32)
            nc.sync.dma_start(res3_sb[:], res3w[:, :])
            wfc_sb = cpool.tile([HID, N_CLS], f32)
            nc.sync.dma_start(wfc_sb[:], wfc[:, :])
            bias_sb = [None] * 4
            for i, d in enumerate(bias_d):
                if d is not None:
                    bias_sb[i] = cpool.tile(list(bias_shapes[i]), f32)
                    nc.sync.dma_start(bias_sb[i][:], d[:, :])

            h2T = bigpool.tile([P, SP], f32)          # for layer-3 residual
            er_sb = [bigpool.tile([P, T, 4], bf16, name=f"er{i}_sb") for i in range(3)]


            def bcast_mid(ap, n):
                return bass.AP(ap.tensor, ap.offset, [ap.ap[0], [0, n], ap.ap[1]])

            def dense_tile(t, lhsT_ap, w_sb, ncols, layer_i):
                """feat_aug for tile t of next layer: write agin + er_sb."""
                ps = p_dn.tile([P, ncols], f32, tag="ps_dense")
                nc.tensor.matmul(ps[:], lhsT=lhsT_ap, rhs=w_sb[:], start=True,
                                 stop=True)
                fsb = wpool.tile([P, P], bf16, tag="fsb")
                nc.scalar.activation(fsb[:, :min(P, ncols)],
                                     ps[:, :min(P, ncols)], Act.Copy)
                nc.sync.dma_start(agin[layer_i][t * P:(t + 1) * P, :],
                                  fsb[:, :P] if ncols >= P else fsb[:])
                if layer_i < 2:
                    nc.vector.tensor_copy(er_sb[layer_i][:, t, :], ps[:, 128:132])
                else:
                    nc.vector.tensor_copy(er_sb[2][:, t, 0:1], ps[:, 33:34])

            # layer-1 dense from xT
            for t in range(T):
                lh = wpool.tile([P, P], f32, tag="xt_t")
                nc.sync.dma_start(lh[:], xT[:, t * P:(t + 1) * P])
                dense_tile(t, lh[:], w1_sb, 132, 0)
            nc.gpsimd.collective_compute(
                "AllGather", Alu.bypass, replica_groups=rg,
                ins=[agin[0][:, :]], outs=[tables[0][:, :]])

            def edge_phase(layer):  # 1-based
                li = layer - 1
                H = 4 if layer < 3 else 1
                FE = H * HID
                table = tables[li]
                act = layer < 3
                for t in range(T):
                    r0 = t * P
                    NCH = nch_t[t]
                    gix = spool.tile([P, ICW], i16, tag="gix")
                    nc.sync.dma_start(gix[:], gidx[r0:r0 + P, :])
                    dof = spool.tile([P, NCHMAX], i16, tag="dof")
                    nc.sync.dma_start(dof[:], dstoff[r0:r0 + P, :])
                    oht_sb = opool.tile([P, NCHMAX, P], bf16, tag="oht")
                    nc.sync.dma_start(oht_sb[:, :NCH, :],
                                      ohtd[r0:r0 + P, :NCH * P])
                    gsb = gpool.tile([P, NCHMAX, P], bf16, tag="gsb")
                    for q in range(NQUARTER):
                        niq = ni_tq[t][q]
                        if niq == 0:
                            continue
                        chq, b0 = ch_tq[t][q], b0_tq[t][q]
                        ic0, icq = icol0_tq[t][q], ic_tq[t][q]
                        if niq % P:
                            # zero the last chunk before gathering over it: the
                            # tail rows stay zero so the masked matmul never
                            # multiplies non-finite stale bits
                            nc.gpsimd.memset(gsb[:, b0 + chq - 1, :], 0.0)
                        nc.gpsimd.dma_gather(
                            gsb[:, b0:b0 + chq, :],
                            table[q * QROWS:(q + 1) * QROWS, :],
                            gix[:, ic0:ic0 + icq],
                            num_idxs=niq, num_idxs_reg=niq, elem_size=P,
                            queue_num=q, single_packet=False,
                        )
                    # one-hot [edges, dst] from dstoff
                    oh = wpool.tile([P, NCHMAX, P], bf16, tag="oh")
                    nc.vector.tensor_tensor(
                        out=oh[:, :NCH, :], in0=bcast_mid(iota_bf[:, :], NCH),
                        in1=dof[:, :NCH].bitcast(bf16).to_broadcast([P, NCH, P]),
                        op=Alu.is_equal)
                    # el on the fly (layers 1-2); layer 3 gathers el directly
                    if act:
                        tmp = wpool.tile([P, NCHMAX, P], bf16, tag="tmp")
                        nc.vector.tensor_tensor(
                            out=tmp[:, :NCH, :], in0=gsb[:, :NCH, :],
                            in1=bcast_mid(al_sb[li][:, :], NCH), op=Alu.mult)
                        elred = spool.tile([P, NCHMAX, H], f32, tag="elred")
                        nc.vector.reduce_sum(
                            elred[:, :NCH, :],
                            tmp[:, :NCH, :].rearrange("p c (h d) -> p c h d", h=H),
                            axis=mybir.AxisListType.X)
                        el_ap = elred[:, :NCH, :]
                    else:
                        el_ap = gsb[:, :NCH, 32:33]
                    # er expansion via host transposed one-hot
                    pse = p_er.tile([P, NCHMAX * H], f32, tag="ps_er")
                    for c in range(NCH):
                        nc.tensor.matmul(
                            pse[:, c * H:(c + 1) * H],
                            lhsT=oht_sb[:, c, :], rhs=er_sb[li][:, t, :H],
                            start=True, stop=True)
                    esb = spool.tile([P, NCHMAX, H], f32, tag="e")
                    nc.vector.tensor_tensor(
                        out=esb[:, :NCH, :], in0=el_ap,
                        in1=pse[:, :NCH * H].rearrange("p (c h) -> p c h", h=H),
                        op=Alu.add)
                    qsb = spool.tile([P, NCHMAX, H], f32, tag="q")
                    nc.vector.tensor_scalar_mul(qsb[:, :NCH, :], esb[:, :NCH, :],
                                                NEG_SLOPE)
                    nc.vector.tensor_tensor(out=esb[:, :NCH, :],
                                            in0=esb[:, :NCH, :],
                                            in1=qsb[:, :NCH, :], op=Alu.max)
                    ex = spool.tile([P, NCHMAX, H], f32, tag="ex")
                    nc.scalar.activation(ex[:, :NCH, :], esb[:, :NCH, :], Act.Exp)
                    # g = [feat*ex | ex]
                    g = wpool.tile([P, NCHMAX, FE + H], bf16, tag="g")
                    nc.vector.tensor_tensor(
                        out=g[:, :NCH, 0:FE].rearrange("p c (h d) -> p c h d", h=H),
                        in0=gsb[:, :NCH, 0:FE].rearrange("p c (h d) -> p c h d", h=H),
                        in1=ex[:, :NCH, :].to_broadcast([P, NCH, H, HID]),
                        op=Alu.mult)
                    nc.vector.tensor_copy(g[:, :NCH, FE:FE + H], ex[:, :NCH, :])
                    # aggregate
                    psa = p_agg.tile([P, FE + H], f32, tag="ps_agg")
                    for c in range(NCH):
                        nc.tensor.matmul(psa[:], lhsT=oh[:, c, :], rhs=g[:, c, :],
                                         start=(c == 0), stop=(c == NCH - 1))
                    # epilogue
                    ssb = spool.tile([P, H], f32, tag="s")
                    nc.vector.tensor_scalar_max(ssb[:], psa[:, FE:FE + H], 1e-30)
                    rec = spool.tile([P, H], f32, tag="rec")
                    nc.vector.reciprocal(rec[:], ssb[:])
                    osb = wpool.tile([P, FE], f32, tag="osb")
                    nc.vector.tensor_tensor(
                        out=osb[:].rearrange("p (h d) -> p h d", h=H),
                        in0=psa[:, 0:FE].rearrange("p (h d) -> p h d", h=H),
                        in1=rec[:].to_broadcast([P, H, HID]), op=Alu.mult)
                    if layer == 2:
                        rsb = wpool.tile([P, P], f32, tag="rsb")
                        nc.sync.dma_start(rsb[:], h1d[r0:r0 + P, :])
                        nc.vector.tensor_tensor(out=osb[:], in0=osb[:],
                                                in1=rsb[:], op=Alu.add)
                    elif layer == 3:
                        psr = p_rs.tile([P, HID], f32, tag="ps_res")
                        nc.tensor.matmul(psr[:], lhsT=h2T[:, r0:r0 + P],
                                         rhs=res3_sb[:], start=True, stop=True)
                        nc.vector.tensor_tensor(out=osb[:], in0=osb[:],
                                                in1=psr[:], op=Alu.add)
                    if bias_sb[li] is not None:
                        nc.vector.tensor_tensor(out=osb[:], in0=osb[:],
                                                in1=bias_sb[li][:, :FE],
                                                op=Alu.add)
                    if act:  # ELU
                        msb = wpool.tile([P, FE], f32, tag="m")
                        nc.vector.tensor_scalar_min(msb[:], osb[:], 0.0)
                        emsb = wpool.tile([P, FE], f32, tag="em")
                        nc.scalar.activation(emsb[:], msb[:], Act.Exp)
                        rlsb = wpool.tile([P, FE], f32, tag="rl")
                        nc.scalar.activation(rlsb[:], osb[:], Act.Relu)
                        nc.vector.tensor_tensor(out=osb[:], in0=rlsb[:],
                                                in1=emsb[:], op=Alu.add)
                        nc.vector.tensor_scalar_add(osb[:], osb[:], -1.0)
                    if layer == 1:
                        nc.sync.dma_start(h1d[r0:r0 + P, :], osb[:])
                    # transpose; feeds next dense / h2T / classifier
                    pst = p_tr.tile([P, P], f32, tag="ps_t")
                    nc.tensor.transpose(pst[:FE, :], osb[:], ident[:])
                    if layer == 1:
                        hts = wpool.tile([P, P], f32, tag="h_t")
                        nc.scalar.activation(hts[:], pst[:], Act.Copy)
                        dense_tile(t, hts[:], w2_sb, 132, 1)
                    elif layer == 2:
                        nc.scalar.activation(h2T[:, r0:r0 + P], pst[:], Act.Copy)
                        dense_tile(t, h2T[:, r0:r0 + P], w3_sb, P, 2)
                    else:
                        hts = spool.tile([HID, P], f32, tag="h3t")
                        nc.scalar.activation(hts[:], pst[:HID, :], Act.Copy)
                        psf = p_dn.tile([P, N_CLS], f32, tag="ps_dense")
                        nc.tensor.matmul(psf[:], lhsT=hts[:], rhs=wfc_sb[:],
                                         start=True, stop=True)
                        ofc = spool.tile([P, N_CLS], f32, tag="ofc")
                        nc.vector.tensor_copy(ofc[:], psf[:])
                        if bias_sb[3] is not None:
                            nc.vector.tensor_tensor(out=ofc[:], in0=ofc[:],
                                                    in1=bias_sb[3][:, :],
                                                    op=Alu.add)
                        rows = min(shard - r0, P)
                        if rows > 0:
                            nc.sync.dma_start(out_e[r0:r0 + rows, :],
                                              ofc[:rows, :])

            edge_phase(1)
            nc.gpsimd.collective_compute(
                "AllGather", Alu.bypass, replica_groups=rg,
                ins=[agin[1][:, :]], outs=[tables[1][:, :]])
            edge_phase(2)
            nc.gpsimd.collective_compute(
                "AllGather", Alu.bypass, replica_groups=rg,
                ins=[agin[2][:, :]], outs=[tables[2][:, :]])
            edge_phase(3)

    nc.compile()
    return nc


def _get_program(n_cores, plan, has_bias):
    key = (n_cores, plan["shard"], plan["NCHMAX"], plan["ICW"],
           tuple(plan["nch_t"]), tuple(map(tuple, plan["ni_tq"])), has_bias)
    if key not in _PROGRAM_CACHE:
        _PROGRAM_CACHE[key] = _build_program(n_cores, plan, has_bias)
    return _PROGRAM_CACHE[key]


def _make_in_maps(prep, inputs, has_bias, n_cores=N_CORES):
    waug1 = _make_waug(inputs["W1"], inputs["al1"], inputs["ar1"], False)
    waug2 = _make_waug(inputs["W2"], inputs["al2"], inputs["ar2"], False)
    waug3 = _make_waug(inputs["W3"], inputs["al3"], inputs["ar3"], True)
    waug3 = np.concatenate(
        [waug3, np.zeros((P, P - waug3.shape[1]), np.float32)], axis=1)

    def al_rep(al):
        return np.ascontiguousarray(
            np.broadcast_to(al.reshape(1, -1), (P, P)).astype(BF16))

    biases = []
    shapes = [(P, P), (P, P), (P, HID), (P, N_CLS)]
    for i, nm in enumerate(("b1", "b2", "b3", "bfc")):
        b = np.asarray(inputs[nm], np.float32).reshape(1, -1)
        biases.append(np.ascontiguousarray(np.broadcast_to(b, shapes[i])))
    in_maps = []
    for c in range(n_cores):
        m = dict(
            xT=prep["xT_per_core"][c],
            gidx=prep["gidx_per_core"][c],
            dstoff=prep["dstoff_per_core"][c],
            ohtd=prep["oht_per_core"][c],
            waug1=waug1, waug2=waug2, waug3=waug3,
            al1r=al_rep(np.asarray(inputs["al1"], np.float32)),
            al2r=al_rep(np.asarray(inputs["al2"], np.float32)),
            res3w=np.asarray(inputs["res3"], np.float32),
            wfc=np.asarray(inputs["Wfc"], np.float32),
        )
        for i, hb in enumerate(has_bias):
            if hb:
                m[f"bias{i}"] = biases[i]
        in_maps.append(m)
    return in_maps


def run_gat(inputs, n_cores=N_CORES, trace=False):
    """Builds (cached), runs on hardware, returns (output, BassKernelResults)."""
    from concourse import bass_utils

    x, src, dst = inputs["x"], inputs["src"], inputs["dst"]
    prep = _prepare(x, src, dst, n_cores)
    has_bias = tuple(
        bool(np.any(np.asarray(inputs[nm]))) for nm in ("b1", "b2", "b3", "bfc"))
    nc = _get_program(n_cores, prep, has_bias)
    in_maps = _make_in_maps(prep, inputs, has_bias, n_cores)
    res = bass_utils.run_bass_kernel_spmd(
        nc, in_maps, core_ids=list(range(n_cores)), trace=trace)
    out = np.concatenate([r["out"] for r in res.results], axis=0)
    return out[: x.shape[0]].astype(np.float32), res


def kernel(**inputs):
    out, _ = run_gat(inputs)
    return out

